# revision 5
# baseline (speedup 1.0000x reference)
# SAGAN self-attention (B=4, H=W=64, C=64, D=8) on 8 TRN2 NeuronCores.
#
# Sharding: core i = (batch b=i//2, half h=i%2); each core computes rows
# [h*2048,(h+1)*2048) of the 4096x4096 attention for its batch, fused in SBUF.
#
# v2 vs baseline: (1) all attention matmuls in fp8e4 — scores at the same
# 1 col/cycle as bf16 but lower PE power (lifts the HAM duty throttle), and
# the PV matmul in DoubleRow perf mode contracting 256 keys/instruction
# (half the PE columns). (2) exp is split across two engines per pair of
# key chunks: ACT runs the real exp (fp8 out), DVE computes the fp8
# BITS directly as round(s*8/ln2 + 56) into a uint8 view (Schraudolph PWL
# exp in e4m3 bit space) — one instruction per element either way, so the
# softmax exp sustains ~2.2 elem/lane/cycle aggregate instead of 1 on ACT.
# (3) loop is pair-grain: 2 score matmuls [128,512] -> one [128,1024] exp
# -> 2 DoubleRow PV matmuls [33,256]; PSUM = 3x2 (scores) + 1 (psv) +
# 1 (epilogue) banks.
#
# Scores stay transposed sT[m,n] = keys on partitions (K=65 contraction
# padded to 128: channels + bias row + zeros); the PV stationary per pair
# is [128, 2, 33] = [hv(8)|0..|1@32] per chunk so one accumulating pass
# yields v_unnorm^T AND the softmax denominator on psum partition 32.
# Epilogue (VD2 pack -> Wv matmul + denominator transpose -> reciprocal ->
# scalar_tensor_tensor residual) identical to baseline.

import numpy as np
import ml_dtypes

import concourse.bacc as bacc
import concourse.tile as tile
import concourse.mybir as mybir
from concourse.alu_op_type import AluOpType
from concourse.bass_utils import run_bass_kernel_spmd

F32 = mybir.dt.float32
BF16 = mybir.dt.bfloat16
FP8 = mybir.dt.float8e4
U8 = mybir.dt.uint8
AFT = mybir.ActivationFunctionType
DR = mybir.MatmulPerfMode.DoubleRow

B, HH, WW, C = 4, 64, 64, 64
N = HH * WW          # 4096 sequence positions per batch
D = 8                # qkv channel dim
RPC = N // 2         # rows per core (2048)
NCORES = 8
MC = N // 128        # 32 key chunks of 128
PAIRS = MC // 2      # 16 chunk pairs (DoubleRow contracts 256 keys)
NT = 4               # n-tiles of 512 query rows
TN = 512

A_EXP = 8.0 / np.log(2.0)        # 11.5416.. : d(bits)/d(s) in e4m3 space
B_EXP = 56.0                     # exponent-bias term; +rounding tweak below
B_RND = 0.0                      # set to +0.5 if engine cast truncates

# engine per pair: A=ACT real exp, D=DVE bits. (Pool/GpSimd cannot read
# PSUM on TRN2, so it can't join the exp split.) Even split; the epilogue
# work lands at pair 0 of the next ntile, which is an 'A' slot, so DVE is
# free for the reciprocal+STT there; the vd2 cast runs on ACT.
SCHED = ['A', 'D'] * 8


def _build():
    nc = bacc.Bacc("TRN2", target_bir_lowering=False, debug=False,
                   num_devices=NCORES)

    xt2 = nc.dram_tensor("xt2", [128, N], FP8, kind="ExternalInput").ap()
    gp2 = nc.dram_tensor("gp2", [128, RPC], FP8, kind="ExternalInput").ap()
    hvo = nc.dram_tensor("hvo", [128, PAIRS * 128], FP8,
                         kind="ExternalInput").ap()
    xrp = nc.dram_tensor("xrp", [128, RPC // 128 * C], F32,
                         kind="ExternalInput").ap()
    wv2 = nc.dram_tensor("wv2", [128, C + 1], BF16,
                         kind="ExternalInput").ap()
    out = nc.dram_tensor("out", [RPC, C], F32, kind="ExternalOutput").ap()

    with tile.TileContext(nc) as tc:
        with tc.tile_pool(name="const", bufs=1) as const:
            XT2 = const.tile([128, N], FP8)
            GP2 = const.tile([128, RPC], FP8)
            HVO = const.tile([128, PAIRS * 128], FP8)
            XRP = const.tile([128, RPC // 128 * C], F32)
            WV2 = const.tile([128, C + 1], BF16)
            PRE = const.tile([1, 1], F32)
            WUP = const.tile([128, 256], FP8)

            # input DMAs in first-use order (pair 0 of ntile 0 first)
            nc.sync.dma_start(GP2[:, 0:512], gp2[:, 0:512])
            hw2 = PAIRS * 128 // 2
            for j in range(8):
                nc.sync.dma_start(XT2[:, j * 512:(j + 1) * 512],
                                  xt2[:, j * 512:(j + 1) * 512])
                if j < 2:
                    nc.sync.dma_start(HVO[:, j * hw2:(j + 1) * hw2],
                                      hvo[:, j * hw2:(j + 1) * hw2])
            for j in range(1, 4):
                nc.sync.dma_start(GP2[:, j * 512:(j + 1) * 512],
                                  gp2[:, j * 512:(j + 1) * 512])
            nc.sync.dma_start(WV2[:], wv2[:])
            nc.sync.dma_start(XRP[:], xrp[:])
            nc.vector.memset(WUP[:], 0.0)
            # dummy exp: hoists the one-time ACT table load into the initial
            # DMA wait instead of the first pair's critical path
            nc.scalar.activation(PRE[:], PRE[0:1, :], AFT.Exp)

            with tc.tile_pool(name="ps_s", bufs=3, space="PSUM") as ps_s, \
                 tc.tile_pool(name="ps_v", bufs=1, space="PSUM") as ps_vp, \
                 tc.tile_pool(name="ps_e", bufs=1, space="PSUM") as ps_ep, \
                 tc.tile_pool(name="expp", bufs=8) as expp, \
                 tc.tile_pool(name="vd2p", bufs=2) as vd2p, \
                 tc.tile_pool(name="scolp", bufs=2) as scolp, \
                 tc.tile_pool(name="osbp", bufs=8) as osbp:
                # PE warm-up during the initial DMA wait lifts the HAM clock
                # throttle before real pairs begin; output scratch, never read
                wps = ps_ep.tile([128, 260], F32, tag="pse")
                for wi in range(16):
                    nc.tensor.matmul(wps[:, 0:256], lhsT=WUP[:, 0:128],
                                     rhs=WUP[:], start=True, stop=True,
                                     skip_group_check=True)

                def epilogue(nt, vd2):
                    pse = ps_ep.tile([128, 260], F32, tag="pse")
                    scol = scolp.tile([128, 4], F32)
                    for nb in range(4):
                        nc.tensor.matmul(pse[:, nb * 65:(nb + 1) * 65],
                                         lhsT=vd2[:, nb * 128:(nb + 1) * 128],
                                         rhs=WV2[:], start=True, stop=True)
                    den4 = pse[:].rearrange("q (nb c) -> q nb c", c=65)
                    nc.vector.reciprocal(scol[:], den4[:, :, 64])
                    for nb in range(4):
                        t = nt * 4 + nb
                        # per-block tiles: tile-granular dep tracking would
                        # otherwise chain the 4 STTs on WAW and stall DVE
                        osb = osbp.tile([128, C], F32)
                        nc.vector.scalar_tensor_tensor(
                            osb[:],
                            pse[:, nb * 65:nb * 65 + 64],
                            scol[:, nb:nb + 1],
                            XRP[:, t * C:(t + 1) * C],
                            op0=AluOpType.mult, op1=AluOpType.add)
                        nc.sync.dma_start(out[t * 128:(t + 1) * 128, :], osb[:])

                pending = None
                for nt in range(NT):
                    n0 = nt * TN
                    psv = ps_vp.tile([64, TN], F32)
                    vd2 = vd2p.tile([128, TN], BF16)
                    if nt < 2:
                        # rows 33-127 are written only by these memsets; with
                        # bufs=2 later n-tiles reuse them still-zeroed
                        nc.gpsimd.memset(vd2[:], 0.0)

                    def emit_pv(hv2, exv, p):
                        for cb in range(2):
                            nc.tensor.matmul(
                                psv[:, cb * 256:(cb + 1) * 256],
                                lhsT=hv2,
                                rhs=exv[:, :, cb * 256:(cb + 1) * 256],
                                start=(p == 0), stop=(p == PAIRS - 1),
                                perf_mode=DR, skip_group_check=True)

                    # PV emission lags the scores by 2 pairs: the in-order PE
                    # stream reaches PV(p) ~2 pair-times after exp(p) was
                    # issued, so the PE never waits on the exp engines even
                    # when they are the pacer (degraded-clock states)
                    pending_pv = []
                    for p in range(PAIRS):
                        ps = ps_s.tile([128, 1024], F32)
                        for j in range(2):
                            m = 2 * p + j
                            nc.tensor.matmul(
                                ps[:, j * 512:(j + 1) * 512],
                                lhsT=XT2[:, m * 128:(m + 1) * 128],
                                rhs=GP2[:, n0:n0 + TN],
                                start=True, stop=True)
                        ex = expp.tile([128, 1024], FP8)
                        eng = SCHED[p]
                        if eng == 'A':
                            nc.scalar.activation(ex[:], ps[:], AFT.Exp)
                        else:
                            nc.vector.tensor_scalar(
                                ex[:].bitcast(U8), ps[:], A_EXP, B_EXP + B_RND,
                                AluOpType.mult, AluOpType.add)
                        exv = ex[:].rearrange("q (k n) -> q k n", k=2)
                        hv2 = HVO[:, p * 128:(p + 1) * 128].rearrange(
                            "q (k m) -> q k m", k=2)
                        pending_pv.append((hv2, exv, p))
                        if len(pending_pv) > 2:
                            emit_pv(*pending_pv.pop(0))
                        # previous n-tile's epilogue one pair late so the
                        # in-order PE queue never stalls on the vd2 cast
                        if p == 0 and pending is not None:
                            epilogue(*pending)
                            pending = None
                    for item in pending_pv:
                        emit_pv(*item)
                    if nt == NT - 1:
                        # final tile: ACT idle since exp(14), DVE free right
                        # after exp(15) gated PV(15) — both halves genuinely
                        # parallel, unlike the exp-split that poisoned v15
                        nc.scalar.activation(vd2[0:33, 0:256],
                                             psv[0:33, 0:256], AFT.Copy)
                        nc.vector.tensor_copy(vd2[0:33, 256:512],
                                              psv[0:33, 256:512])
                    else:
                        nc.scalar.activation(vd2[0:33, :], psv[0:33, :],
                                             AFT.Copy)
                    pending = (nt, vd2)
                epilogue(*pending)
    nc.compile()
    return nc


_CACHE = {}


def _get_compiled():
    if "nc" not in _CACHE:
        _CACHE["nc"] = _build()
    return _CACHE["nc"]


def _make_in_maps(x, Wf, bf, Wg, bg, Wh, bh, Wv, bv, gamma):
    x = np.asarray(x, np.float32)
    Wf = np.asarray(Wf, np.float32)
    Wg = np.asarray(Wg, np.float32)
    Wh = np.asarray(Wh, np.float32)
    Wv = np.asarray(Wv, np.float32)
    bf = np.asarray(bf, np.float32)
    bg = np.asarray(bg, np.float32)
    bh = np.asarray(bh, np.float32)
    bv = np.asarray(bv, np.float32)
    g0 = float(np.asarray(gamma, np.float32).reshape(-1)[0])

    f8 = ml_dtypes.float8_e4m3
    xf = x.reshape(B, N, C)
    P = Wf @ Wg.T                            # [C, C] score kernel
    wfbg = Wf @ bg                           # [C] column-bias direction
    bgbf = float(bg @ bf)
    res_bias = g0 * (bh @ Wv + bv)           # [C] folded into residual
    wv2 = np.zeros((128, C + 1), np.float32)
    wv2[0:D, 0:C] = g0 * Wv
    wv2[32, C] = 1.0                         # denominator extraction column
    wv2 = wv2.astype(ml_dtypes.bfloat16)

    in_maps = []
    for i in range(NCORES):
        b, h = divmod(i, 2)
        r0 = h * RPC
        xt2 = np.zeros((128, N), np.float32)
        xt2[0:C] = xf[b].T
        xt2[C] = xf[b] @ wfbg + bgbf         # d_m: per-key score bias
        gp2 = np.zeros((128, RPC), np.float32)
        gp2[0:C] = P @ xf[b, r0:r0 + RPC].T
        gp2[C] = 1.0
        hv = xf[b] @ Wh                      # [N, D] (bh folds into res_bias)
        ho = np.zeros((PAIRS, 2, 128, 64), np.float32)
        ho[:, :, :, 0:D] = hv.reshape(PAIRS, 2, 128, D)
        ho[:, :, :, 32] = 1.0
        # [pair, sub, part, 64] -> [part, pair, sub, 64] -> [128, PAIRS*128]
        ho = np.ascontiguousarray(ho.transpose(2, 0, 1, 3).reshape(128, -1))
        xr = xf[b, r0:r0 + RPC] + res_bias   # [RPC, C]
        xrp = np.ascontiguousarray(
            xr.reshape(RPC // 128, 128, C).transpose(1, 0, 2).reshape(128, -1))
        in_maps.append({"xt2": xt2.astype(f8),
                        "gp2": gp2.astype(f8),
                        "hvo": ho.astype(f8),
                        "xrp": xrp, "wv2": wv2})
    return in_maps


def _assemble(results):
    outf = np.empty((B, N, C), np.float32)
    for i in range(NCORES):
        b, h = divmod(i, 2)
        outf[b, h * RPC:(h + 1) * RPC] = results[i]["out"]
    return outf.reshape(B, HH, WW, C)


def run(inputs, **spmd_kwargs):
    """Returns (output, BassKernelResults)."""
    nc = _get_compiled()
    in_maps = _make_in_maps(**inputs)
    res = run_bass_kernel_spmd(nc, in_maps, core_ids=list(range(NCORES)),
                               **spmd_kwargs)
    return _assemble(res.results), res


def kernel(**inputs):
    out, _ = run(inputs)
    return out
